# revision 2
# baseline (speedup 1.0000x reference)
"""Bass/Trainium2 kernel for nn_CustomLoss_87952340287807.

Loss over B=8,388,608 Euler-angle triples:
    per-sample = 1 - |cross(vo, vt)| + dot(vo, vt),  summed.
vo/vt are unit vectors, so |cross| = sqrt(1 - dot^2) and only dot is needed.

dot = cosD*(u*U + v*V) + sinD*(u*V - v*U) + (cp*CP)*(cr*CR)
  u = sin(p~)cos(r~), v = sin(r~)   (o side; caps = t side)
  D = 2*pi*(yt - yo)
All trig from the Sin LUT (valid domain [-pi, pi]):
  sin(2pi(x-.5)) = Sin(2pi*x - pi)
  cos(2pi(x-.5)) = 1 - 2*h^2,  h = Sin(pi*x - pi/2)
  cosD = 1 - 2*hD^2, sinD = hD*(2 - 4*jD^2);  hD = Sin(pi*e), jD = Sin(pi*e/2)

Engine split per tile: ScalarE does all LUT evals (strided reads are free
there); DVE does the bilinear chain; GpSimd takes 4 ops (cp*CP, cr*CR,
clamp, q=hc^2); one deferred Sqrt pass (separate ACT table) computes
|cross| for the whole core with a fused accumulator.

Sharding: pure data-parallel, batch split across 8 NeuronCores; each core
returns per-partition partial sums of dot and cross-norm; host reduces.
"""
import sys

import numpy as np

if "/opt/trn_rl_repo" not in sys.path:
    sys.path.insert(0, "/opt/trn_rl_repo")

import concourse.bacc as bacc
import concourse.mybir as mybir
import concourse.tile as tile
from concourse.bass_utils import run_bass_kernel_spmd

B = 8388608
NCORES = 8
S = B // NCORES          # 1,048,576 samples per core
P = 128
F = 2048                 # samples per partition per tile
NT = S // (P * F)        # 4 tiles

AF = mybir.ActivationFunctionType
ALU = mybir.AluOpType
dt = mybir.dt
f32, f16 = dt.float32, dt.float16
PI = float(np.pi)

GPS_X = True             # cp*CP, cr*CR on gpsimd
GPS_TAIL = True          # clamp + square on gpsimd

_cache = {}
last_results = None


def _build():
    nc = bacc.Bacc("TRN2", target_bir_lowering=False, debug=False)
    o_in = nc.declare_dram_parameter("out_in", [S, 3], f32, isOutput=False)
    t_in = nc.declare_dram_parameter("tgt_in", [S, 3], f32, isOutput=False)
    res = nc.declare_dram_parameter("res", [P, NT + 1], f32, isOutput=True)

    o_flat = o_in.ap().rearrange("(p n) c -> p (n c)", p=P)
    t_flat = t_in.ap().rearrange("(p n) c -> p (n c)", p=P)

    with tile.TileContext(nc) as tc:
        with tc.tile_pool(name="consts", bufs=1) as cpool, \
             tc.tile_pool(name="raw", bufs=2) as rawpool, \
             tc.tile_pool(name="sb", bufs=1) as pool, \
             tc.tile_pool(name="persist", bufs=1) as ppool:
            consts = {}
            for i, val in enumerate([-PI, -PI / 2, 1.0]):
                ct = cpool.tile([P, 1], f32, name=f"cst{i}", tag=f"cst{i}")
                nc.vector.memset(ct[:], val)
                consts[val] = ct[:]

            q_all = ppool.tile([P, NT * F], f16, name="q_all", tag="q_all")
            cn_all = ppool.tile([P, NT * F], f16, name="cn_all", tag="cn_all")
            dacc = ppool.tile([P, NT], f32, name="dacc", tag="dacc")
            cacc = ppool.tile([P, 1], f32, name="cacc", tag="cacc")

            def mk(tag, cols=F, d=f16):
                return pool.tile([P, cols], d, name=tag, tag=tag)

            def load(i):
                ro = rawpool.tile([P, 3 * F], f16, name="raw_o", tag="raw_o")
                nc.gpsimd.dma_start(ro[:], o_flat[:, i * 3 * F:(i + 1) * 3 * F])
                rt = rawpool.tile([P, 3 * F], f16, name="raw_t", tag="raw_t")
                nc.gpsimd.dma_start(rt[:], t_flat[:, i * 3 * F:(i + 1) * 3 * F])
                return ro, rt

            eng_x = nc.gpsimd if GPS_X else nc.vector
            eng_t = nc.gpsimd if GPS_TAIL else nc.vector

            raws = load(0)
            for i in range(NT):
                raw_o, raw_t = raws
                if i + 1 < NT:
                    raws = load(i + 1)

                ov = raw_o[:].rearrange("p (n c) -> p c n", c=3)
                tv = raw_t[:].rearrange("p (n c) -> p c n", c=3)
                yo, yt = ov[:, 0, :], tv[:, 0, :]
                pr_o, pr_t = ov[:, 1:3, :], tv[:, 1:3, :]

                # full-angle sines [sp | sr] and half-angle sines [hp | hr]
                sf_o = mk("sf_o", 2 * F)
                nc.scalar.activation(sf_o[:].rearrange("p (c n) -> p c n", c=2),
                                     pr_o, AF.Sin, bias=consts[-PI], scale=2 * PI)
                hh_o = mk("hh_o", 2 * F)
                nc.scalar.activation(hh_o[:].rearrange("p (c n) -> p c n", c=2),
                                     pr_o, AF.Sin, bias=consts[-PI / 2], scale=PI)
                sf_t = mk("sf_t", 2 * F)
                nc.scalar.activation(sf_t[:].rearrange("p (c n) -> p c n", c=2),
                                     pr_t, AF.Sin, bias=consts[-PI], scale=2 * PI)
                hh_t = mk("hh_t", 2 * F)
                nc.scalar.activation(hh_t[:].rearrange("p (c n) -> p c n", c=2),
                                     pr_t, AF.Sin, bias=consts[-PI / 2], scale=PI)

                # yaw delta and its half/quarter sines
                e = mk("e")
                nc.vector.tensor_sub(e[:], yt, yo)
                hD = mk("hD")
                nc.scalar.activation(hD[:], e[:], AF.Sin, scale=PI)
                jD = mk("jD")
                nc.scalar.activation(jD[:], e[:], AF.Sin, scale=PI / 2)

                # cos pairs: c = 1 - 2h^2  ([cp | cr] blocks)
                qq_o = mk("qq_o", 2 * F)
                nc.vector.tensor_mul(qq_o[:], hh_o[:], hh_o[:])
                cc_o = mk("hh_o", 2 * F)
                nc.vector.tensor_scalar(cc_o[:], qq_o[:], -2.0, 1.0,
                                        ALU.mult, ALU.add)
                qq_t = mk("qq_t", 2 * F)
                nc.vector.tensor_mul(qq_t[:], hh_t[:], hh_t[:])
                cc_t = mk("hh_t", 2 * F)
                nc.vector.tensor_scalar(cc_t[:], qq_t[:], -2.0, 1.0,
                                        ALU.mult, ALU.add)

                sp_o, sr_o = sf_o[:, :F], sf_o[:, F:]
                sp_t, sr_t = sf_t[:, :F], sf_t[:, F:]
                cp_o, cr_o = cc_o[:, :F], cc_o[:, F:]
                cp_t, cr_t = cc_t[:, :F], cc_t[:, F:]

                # gpsimd side-chain: g = (cp*CP)*(cr*CR) = w*W
                x1 = mk("x1")
                eng_x.tensor_mul(x1[:], cp_o, cp_t)
                x2 = mk("x2")
                eng_x.tensor_mul(x2[:], cr_o, cr_t)

                # bilinear chain on DVE
                u = mk("u")
                nc.vector.tensor_mul(u[:], sp_o, cr_o)
                U_ = mk("U_")
                nc.vector.tensor_mul(U_[:], sp_t, cr_t)
                m1 = mk("m1")
                nc.vector.tensor_mul(m1[:], u[:], U_[:])
                m2 = mk("m2")
                nc.vector.tensor_mul(m2[:], sr_o, sr_t)
                a = mk("a")
                nc.vector.tensor_add(a[:], m1[:], m2[:])
                m3 = mk("m3")
                nc.vector.tensor_mul(m3[:], u[:], sr_t)
                m4 = mk("m4")
                nc.vector.tensor_mul(m4[:], sr_o, U_[:])
                b = mk("m1")
                nc.vector.tensor_sub(b[:], m3[:], m4[:])

                g = mk("g")
                nc.vector.tensor_mul(g[:], x1[:], x2[:])

                # D rotation: cD = 1-2qD, sD*b = hD*(2-4qj)*b
                qD = mk("qD")
                nc.vector.tensor_mul(qD[:], hD[:], hD[:])
                qj = mk("qj")
                nc.vector.tensor_mul(qj[:], jD[:], jD[:])
                cD = mk("jD")
                nc.vector.tensor_scalar(cD[:], qD[:], -2.0, 1.0,
                                        ALU.mult, ALU.add)
                t2 = mk("e")
                nc.vector.tensor_scalar(t2[:], qj[:], -4.0, 2.0,
                                        ALU.mult, ALU.add)
                p1 = mk("qD")
                nc.vector.tensor_mul(p1[:], cD[:], a[:])
                t2b = mk("qj")
                nc.vector.tensor_mul(t2b[:], t2[:], b[:])
                q1 = mk("m2")
                nc.vector.tensor_mul(q1[:], hD[:], t2b[:])
                s = mk("u")
                nc.vector.tensor_add(s[:], p1[:], q1[:])

                # dot = g + s, with fused per-partition sum into dacc[:, i]
                dot = mk("m3")
                nc.vector.scalar_tensor_tensor(
                    dot[:], g[:], 0.0, s[:], ALU.add, ALU.add,
                    accum_out=dacc[:, i:i + 1])

                # clamp and square -> q_all chunk (gpsimd tail)
                hc = mk("m4")
                eng_t.tensor_scalar(hc[:], dot[:], -1.0, 1.0, ALU.max, ALU.min)
                eng_t.tensor_mul(q_all[:, i * F:(i + 1) * F], hc[:], hc[:])

            # one deferred sqrt pass (sqrt table loads once):
            # cn = sqrt(1 - q), accumulated per partition
            nc.scalar.activation(cn_all[:], q_all[:], AF.Sqrt,
                                 bias=consts[1.0], scale=-1.0,
                                 accum_out=cacc[:])

            nc.sync.dma_start(res[:, 0:NT], dacc[:])
            nc.sync.dma_start(res[:, NT:NT + 1], cacc[:])

    nc.compile()
    return nc


def kernel(output: np.ndarray, target: np.ndarray) -> np.ndarray:
    global last_results
    if "nc" not in _cache:
        _cache["nc"] = _build()
    nc = _cache["nc"]

    output = np.ascontiguousarray(output, dtype=np.float32)
    target = np.ascontiguousarray(target, dtype=np.float32)
    in_maps = [
        {"out_in": output[c * S:(c + 1) * S], "tgt_in": target[c * S:(c + 1) * S]}
        for c in range(NCORES)
    ]
    r = run_bass_kernel_spmd(nc, in_maps, list(range(NCORES)))
    last_results = r

    total = np.float64(B)
    for c in range(NCORES):
        out = r.results[c]["res"].astype(np.float64)
        total += out[:, 0:NT].sum() - out[:, NT].sum()
    return np.float32(total)


# revision 3
# speedup vs baseline: 1.9978x; 1.9978x over previous
"""Bass/Trainium2 kernel for nn_CustomLoss_87952340287807.

Loss over B=8,388,608 Euler-angle triples:
    per-sample = 1 - |cross(vo, vt)| + dot(vo, vt),  summed.
vo/vt are unit vectors, so |cross| = sqrt(1 - dot^2) and only dot is needed.

dot = cosD*(u*U + v*V) + sinD*(u*V - v*U) + (cp*CP)*(cr*CR)
  u = sin(p~)cos(r~), v = sin(r~)   (o side; caps = t side)
  D = 2*pi*(yt - yo)
All trig from the Sin LUT (valid domain [-pi, pi]):
  sin(2pi(x-.5)) = Sin(2pi*x - pi)
  cos(2pi(x-.5)) = 1 - 2*h^2,  h = Sin(pi*x - pi/2)
  cosD = 1 - 2*hD^2, sinD = hD*(2 - 4*jD^2);  hD = Sin(pi*e), jD = Sin(pi*e/2)

Engine split: ScalarE does all LUT evals + the two r-side squares (Square
is a filler entry in every ACT table set, so no extra table load); DVE
runs the bilinear chain with three fused custom-DVE ops:
  COSPROD:  (1-2a^2)(1-2b^2)   -> cp*CP in one instruction
  COSMUL:   (1-2a^2)*t         -> cosD*a in one instruction
  SINMUL:   (s0-s1*a^2)*t      -> (2-4jD^2)*b in one instruction
One deferred Sqrt pass (separate ACT table) computes |cross| for the
whole core with a fused accumulator.

Sharding: pure data-parallel, batch split across 8 NeuronCores; each core
returns per-partition partial sums of dot and cross-norm; host reduces.
"""
import sys

import numpy as np

if "/opt/trn_rl_repo" not in sys.path:
    sys.path.insert(0, "/opt/trn_rl_repo")

import concourse.bacc as bacc
import concourse.mybir as mybir
import concourse.tile as tile
from concourse import dve_ops as dvo
from concourse.bass_utils import run_bass_kernel_spmd
from concourse.dve_spec import C0, C1, One, Spec, Src0, Src1, _has_src1, lower
from concourse.dve_spec import sq
from concourse.dve_uop import DveOpSpec

B = 8388608
NCORES = 8
S = B // NCORES          # 1,048,576 samples per core
P = 128
F = 2048                 # samples per partition per tile
NT = S // (P * F)        # 4 tiles

AF = mybir.ActivationFunctionType
ALU = mybir.AluOpType
dt = mybir.dt
f32, f16 = dt.float32, dt.float16
PI = float(np.pi)

_cache = {}
last_results = None


def _reg(name, spec):
    """Register a custom DVE op at runtime (per-NEFF table, no firmware
    change). Computes the pinned uops sha the same way DveOp.compile does."""
    for op in dvo.OPS:
        if op.name == name:
            return op
    row = dvo._CUSTOM_DVE_ROW_BASE + len(dvo.OPS)
    assert row < 0x20, "custom-DVE opcode rows exhausted"
    ver = "v3"  # TRN2
    uops = lower(spec, ver=ver)
    sha = DveOpSpec(name=name, opcode=row, uops=uops,
                    rd1_en=_has_src1(spec)).sha(ver)
    op = dvo.DveOp(name, spec, subdim=False, uops_sha={ver: sha})
    dvo.OPS.append(op)
    dvo._SUB_OPCODE_FOR_NAME[name] = row
    dvo.CUSTOM_DVE_SPECS[name] = spec
    return op


# (1 - s0/2... s0=2: (1-2*Src0^2) * (1-2*Src1^2) = cosA*cosB from half-sines
COSPROD = _reg("COSPROD_ANT", Spec(
    body=(One - sq(Src0) * C0) * (One - sq(Src1) * C0)))
# (1-2*Src0^2) * Src1 = cosA * t from half-sine
COSMUL = _reg("COSMUL_ANT", Spec(
    body=(One - sq(Src0) * C0) * Src1))
# (s0 - s1*Src0^2) * Src1; s0=2, s1=4: (2-4*jD^2)*b = (sinD/hD)*b
SINMUL = _reg("SINMUL_ANT", Spec(
    body=(C0 - sq(Src0) * C1) * Src1))


def _build():
    nc = bacc.Bacc("TRN2", target_bir_lowering=False, debug=False)
    o_in = nc.declare_dram_parameter("out_in", [S, 3], f32, isOutput=False)
    t_in = nc.declare_dram_parameter("tgt_in", [S, 3], f32, isOutput=False)
    res = nc.declare_dram_parameter("res", [P, NT + 1], f32, isOutput=True)

    o_flat = o_in.ap().rearrange("(p n) c -> p (n c)", p=P)
    t_flat = t_in.ap().rearrange("(p n) c -> p (n c)", p=P)

    with tile.TileContext(nc) as tc:
        with tc.tile_pool(name="consts", bufs=1) as cpool, \
             tc.tile_pool(name="raw", bufs=2) as rawpool, \
             tc.tile_pool(name="sb", bufs=1) as pool, \
             tc.tile_pool(name="persist", bufs=1) as ppool:
            consts = {}
            for i, val in enumerate([-PI, -PI / 2, 1.0]):
                ct = cpool.tile([P, 1], f32, name=f"cst{i}", tag=f"cst{i}")
                nc.vector.memset(ct[:], val)
                consts[val] = ct[:]

            q_all = ppool.tile([P, NT * F], f16, name="q_all", tag="q_all")
            cn_all = ppool.tile([P, NT * F], f16, name="cn_all", tag="cn_all")
            dacc = ppool.tile([P, NT], f32, name="dacc", tag="dacc")
            cacc = ppool.tile([P, 1], f32, name="cacc", tag="cacc")

            def mk(tag, cols=F, d=f16):
                return pool.tile([P, cols], d, name=tag, tag=tag)

            def load(i):
                ro = rawpool.tile([P, 3 * F], f16, name="raw_o", tag="raw_o")
                nc.gpsimd.dma_start(ro[:], o_flat[:, i * 3 * F:(i + 1) * 3 * F])
                rt = rawpool.tile([P, 3 * F], f16, name="raw_t", tag="raw_t")
                nc.gpsimd.dma_start(rt[:], t_flat[:, i * 3 * F:(i + 1) * 3 * F])
                return ro, rt

            raws = load(0)
            for i in range(NT):
                raw_o, raw_t = raws
                if i + 1 < NT:
                    raws = load(i + 1)

                ov = raw_o[:].rearrange("p (n c) -> p c n", c=3)
                tv = raw_t[:].rearrange("p (n c) -> p c n", c=3)
                yo, yt = ov[:, 0, :], tv[:, 0, :]
                pr_o, pr_t = ov[:, 1:3, :], tv[:, 1:3, :]

                # full-angle sines [sp | sr] and half-angle sines [hp | hr]
                sf_o = mk("sf_o", 2 * F)
                nc.scalar.activation(sf_o[:].rearrange("p (c n) -> p c n", c=2),
                                     pr_o, AF.Sin, bias=consts[-PI], scale=2 * PI)
                hh_o = mk("hh_o", 2 * F)
                nc.scalar.activation(hh_o[:].rearrange("p (c n) -> p c n", c=2),
                                     pr_o, AF.Sin, bias=consts[-PI / 2], scale=PI)
                sf_t = mk("sf_t", 2 * F)
                nc.scalar.activation(sf_t[:].rearrange("p (c n) -> p c n", c=2),
                                     pr_t, AF.Sin, bias=consts[-PI], scale=2 * PI)
                hh_t = mk("hh_t", 2 * F)
                nc.scalar.activation(hh_t[:].rearrange("p (c n) -> p c n", c=2),
                                     pr_t, AF.Sin, bias=consts[-PI / 2], scale=PI)

                # r-side squares on ScalarE (Square is in every table set)
                qro = mk("qro")
                nc.scalar.activation(qro[:], hh_o[:, F:2 * F], AF.Square)
                qrt = mk("qrt")
                nc.scalar.activation(qrt[:], hh_t[:, F:2 * F], AF.Square)

                # yaw delta and its half/quarter sines
                e = mk("e")
                nc.vector.tensor_sub(e[:], yt, yo)
                hD = mk("hD")
                nc.scalar.activation(hD[:], e[:], AF.Sin, scale=PI)
                jD = mk("jD")
                nc.scalar.activation(jD[:], e[:], AF.Sin, scale=PI / 2)

                sp_o, sr_o = sf_o[:, :F], sf_o[:, F:]
                sp_t, sr_t = sf_t[:, :F], sf_t[:, F:]
                hp_o, hp_t = hh_o[:, :F], hh_t[:, :F]

                # cr = 1-2*qro (cos roll); x1 = cp*CP fused from half-sines
                cr_o = mk("cr_o")
                nc.vector.tensor_scalar(cr_o[:], qro[:], -2.0, 1.0,
                                        ALU.mult, ALU.add)
                cr_t = mk("cr_t")
                nc.vector.tensor_scalar(cr_t[:], qrt[:], -2.0, 1.0,
                                        ALU.mult, ALU.add)
                x1 = mk("x1")
                nc.vector._custom_dve(COSPROD, out=x1[:], in0=hp_o, in1=hp_t,
                                      s0=2.0)
                x2 = mk("x2")
                nc.vector.tensor_mul(x2[:], cr_o[:], cr_t[:])
                g = mk("g")
                nc.vector.tensor_mul(g[:], x1[:], x2[:])

                # bilinear chain
                u = mk("u")
                nc.vector.tensor_mul(u[:], sp_o, cr_o[:])
                U_ = mk("U_")
                nc.vector.tensor_mul(U_[:], sp_t, cr_t[:])
                m1 = mk("m1")
                nc.vector.tensor_mul(m1[:], u[:], U_[:])
                m2 = mk("m2")
                nc.vector.tensor_mul(m2[:], sr_o, sr_t)
                a = mk("a")
                nc.vector.tensor_add(a[:], m1[:], m2[:])
                m3 = mk("m3")
                nc.vector.tensor_mul(m3[:], u[:], sr_t)
                m4 = mk("m4")
                nc.vector.tensor_mul(m4[:], sr_o, U_[:])
                b = mk("m1")
                nc.vector.tensor_sub(b[:], m3[:], m4[:])

                # rotation by D: p1 = cosD*a, q1 = sinD*b (fused)
                p1 = mk("u")
                nc.vector._custom_dve(COSMUL, out=p1[:], in0=hD[:], in1=a[:],
                                      s0=2.0)
                t2b = mk("m2")
                nc.vector._custom_dve(SINMUL, out=t2b[:], in0=jD[:], in1=b[:],
                                      s0=2.0, s1=4.0)
                q1 = mk("m3")
                nc.vector.tensor_mul(q1[:], hD[:], t2b[:])
                s = mk("m4")
                nc.vector.tensor_add(s[:], p1[:], q1[:])

                # dot = g + s, with fused per-partition sum into dacc[:, i]
                dot = mk("a")
                nc.vector.scalar_tensor_tensor(
                    dot[:], g[:], 0.0, s[:], ALU.add, ALU.add,
                    accum_out=dacc[:, i:i + 1])

                # clamp and square -> q_all chunk
                hc = mk("m1")
                nc.vector.tensor_scalar(hc[:], dot[:], -1.0, 1.0,
                                        ALU.max, ALU.min)
                nc.vector.tensor_mul(q_all[:, i * F:(i + 1) * F], hc[:], hc[:])

            # one deferred sqrt pass (sqrt table loads once):
            # cn = sqrt(1 - q), accumulated per partition
            nc.scalar.activation(cn_all[:], q_all[:], AF.Sqrt,
                                 bias=consts[1.0], scale=-1.0,
                                 accum_out=cacc[:])

            nc.sync.dma_start(res[:, 0:NT], dacc[:])
            nc.sync.dma_start(res[:, NT:NT + 1], cacc[:])

    nc.compile()
    return nc


def kernel(output: np.ndarray, target: np.ndarray) -> np.ndarray:
    global last_results
    if "nc" not in _cache:
        _cache["nc"] = _build()
    nc = _cache["nc"]

    output = np.ascontiguousarray(output, dtype=np.float32)
    target = np.ascontiguousarray(target, dtype=np.float32)
    in_maps = [
        {"out_in": output[c * S:(c + 1) * S], "tgt_in": target[c * S:(c + 1) * S]}
        for c in range(NCORES)
    ]
    r = run_bass_kernel_spmd(nc, in_maps, list(range(NCORES)))
    last_results = r

    total = np.float64(B)
    for c in range(NCORES):
        out = r.results[c]["res"].astype(np.float64)
        total += out[:, 0:NT].sum() - out[:, NT].sum()
    return np.float32(total)
